# revision 17
# baseline (speedup 1.0000x reference)
"""Trainium2 Bass kernel for CrossAttention (B=4, N=M=2048, H=8, D=64,
Q_DIM=1024, C_DIM=768).

Sharding over 8 cores: core c handles batch b = c//2 and head-group
hg = c%2 (4 heads, 256 inner dims).  Each core computes a *partial*
output projection (its 256 inner dims of the 512 total); the host sums
core pairs and adds the output bias.

Device-side layouts are all matmul-native (out = lhsT.T @ rhs):
  - qT/kT [inner, seq]: computed with weight-chunk stationary, xT/ctxT
    moving.  v [keys, inner] with ctxT-chunk stationary, WvT moving.
  - scores are computed transposed: S.T[keys, q] = kT_h contracted with
    qT_h over the head dim, so softmax's key-reduction is a matmul
    reduction and no on-chip transposes are needed anywhere.
  - V carries an appended ones-column, so the P.T@V matmul also yields the
    per-query softmax denominators (row 64 of the [65, q] accumulator).
  - denominators are broadcast across partitions with a K=1 ones matmul,
    reciprocal'd on VectorE, and folded into the PSUM->SBUF copy of the
    attention output.
  - all matmul inputs are float32r (single-pass FP22 multiply, fp32
    accumulate) for 4x tensor-engine throughput vs true fp32.
  - the attention inner loop is software-pipelined at emission so the
    next chunk's score matmuls sit ahead of the current chunk's PV
    matmul in the PE queue (PV waits on exp; the scores don't).

The attention mask in this problem is all-True; if a mask with False
entries is ever passed, kernel() falls back to a numpy reference.
"""

import numpy as np

B, N, M = 4, 2048, 2048
Q_DIM, C_DIM, H, D = 1024, 768, 8, 64
INNER = H * D  # 512
SCALE = D ** -0.5

N_CORES = 8
H_PER_CORE = 4          # heads per core
IN_PER_CORE = H_PER_CORE * D  # 256 inner dims per core
QB = 1024               # query block
N_QB = N // QB          # 2
KC = M // 128           # 16 key chunks
QK_CHUNKS = Q_DIM // 128   # 8
CK_CHUNKS = C_DIM // 128   # 6
IN_CHUNKS = IN_PER_CORE // 128  # 2

_CACHED_NC = None


def _build_bass():
    import concourse.bass as bass
    import concourse.mybir as mybir
    import concourse.tile as tile
    from concourse import bacc

    f32r = mybir.dt.float32r
    f32 = mybir.dt.float32
    ts, ds = bass.ts, bass.ds
    Exp = mybir.ActivationFunctionType.Exp

    nc = bacc.Bacc("TRN2", target_bir_lowering=False)

    xT = nc.dram_tensor("xT", [Q_DIM, N], f32r, kind="ExternalInput")
    cT = nc.dram_tensor("cT", [C_DIM, M], f32r, kind="ExternalInput")
    wq = nc.dram_tensor("wq", [Q_DIM, IN_PER_CORE], f32r, kind="ExternalInput")
    wk = nc.dram_tensor("wk", [C_DIM, IN_PER_CORE], f32r, kind="ExternalInput")
    wv = nc.dram_tensor("wv", [C_DIM, IN_PER_CORE], f32r, kind="ExternalInput")
    wo = nc.dram_tensor("wo", [IN_PER_CORE, Q_DIM], f32r, kind="ExternalInput")
    out_d = nc.dram_tensor("out", [N, Q_DIM], f32, kind="ExternalOutput")

    with tile.TileContext(nc) as tc:
        with (
            tc.tile_pool(name="persist", bufs=1) as persist,
            tc.tile_pool(name="stream", bufs=2) as stream,
            tc.tile_pool(name="psA", bufs=2, space="PSUM") as psA,
            tc.tile_pool(name="psB", bufs=2, space="PSUM") as psB,
        ):
            # ---- resident weights / constants ----
            # DMA order matters: the kT phase needs wk + ctx first.
            wk_sb = persist.tile([128, CK_CHUNKS, IN_PER_CORE], f32r, tag="wk")
            nc.sync.dma_start(out=wk_sb, in_=wk.rearrange("(k p) n -> p k n", p=128))

            # row D used as the lhsT of the denominator-broadcast matmul; it
            # must sit at the same base partition as the denominator row
            # (partition D of the attention accumulator).  memset can't target
            # float32r, so fill an fp32 scratch and copy-convert.
            onesf = persist.tile([128, D], f32, tag="onesf")
            nc.vector.memset(onesf, 1.0)
            ones_sb = persist.tile([D + 1, D], f32r, tag="ones")
            nc.vector.tensor_copy(out=ones_sb, in_=onesf[0 : D + 1, :])

            # context resident (read by both kT and v phases); per-chunk DMAs
            # so the kT matmuls can start as soon as the first chunk lands.
            ctx_sb = persist.tile([128, CK_CHUNKS, M], f32r, tag="ctx")
            for k in range(CK_CHUNKS):
                nc.sync.dma_start(out=ctx_sb[:, k, :], in_=cT[ds(k * 128, 128), :])

            wq_sb = persist.tile([128, QK_CHUNKS, IN_PER_CORE], f32r, tag="wq")
            nc.sync.dma_start(out=wq_sb, in_=wq.rearrange("(k p) n -> p k n", p=128))

            # load order: x0 right after wq (gates the first exp), then wv
            # (gates the deferred v chunks), wo, and x1 last (needed ~100us in;
            # it waits on the x slot release semaphore at runtime anyway).
            def emit_x_load(qb, chunked=False):
                x_sb = stream.tile([128, QK_CHUNKS, QB], f32r, tag="x", bufs=1,
                                   name=f"x{qb}")
                if chunked:
                    # per-chunk DMAs so the first qT pass paces with arrival
                    for k in range(QK_CHUNKS):
                        nc.sync.dma_start(
                            out=x_sb[:, k, :],
                            in_=xT[ds(k * 128, 128), ds(qb * QB, QB)],
                        )
                else:
                    nc.sync.dma_start(
                        out=x_sb,
                        in_=xT[:, ds(qb * QB, QB)].rearrange("(k p) q -> p k q",
                                                             p=128),
                    )
                return x_sb

            x_tiles = [emit_x_load(0, chunked=True)]

            wv_sb = persist.tile([128, CK_CHUNKS, IN_PER_CORE], f32r, tag="wv")
            nc.sync.dma_start(out=wv_sb, in_=wv.rearrange("(k p) n -> p k n", p=128))
            wo_sb = persist.tile([128, IN_CHUNKS, Q_DIM], f32r, tag="wo")
            nc.sync.dma_start(out=wo_sb, in_=wo.rearrange("(t p) n -> p t n", p=128))
            x_tiles.append(emit_x_load(1))

            # ---- kT [IN_PER_CORE, M] interleaved with qT(qb0) so the PE can
            # alternate between them as the ctx / x0 DMAs land (kps and qps
            # occupy the two A slots concurrently).
            kT_sb = persist.tile([128, IN_CHUNKS, M], f32r, tag="kt")
            qT0_sb = stream.tile([128, IN_CHUNKS, QB], f32r, tag="qt", bufs=1,
                                 name="qT0")

            def emit_kt(m, half):
                kps = psA.tile([128, 1024], f32, tag="A", name=f"kps{m}{half}")
                for k in range(CK_CHUNKS):
                    for j in range(2):
                        nc.tensor.matmul(
                            kps[:, ts(j, 512)],
                            wk_sb[:, k, ts(m, 128)],
                            ctx_sb[:, k, ds(half * 1024 + j * 512, 512)],
                            start=(k == 0),
                            stop=(k == CK_CHUNKS - 1),
                        )
                nc.vector.tensor_copy(
                    out=kT_sb[:, m, ds(half * 1024, 1024)], in_=kps
                )

            def emit_qt(qT_sb, x_sb, m, qb, pool=None, tag="A"):
                pool = pool if pool is not None else psA
                qps = pool.tile([128, QB], f32, tag=tag, name=f"qps{qb}{m}")
                for k in range(QK_CHUNKS):
                    for j in range(2):
                        nc.tensor.matmul(
                            qps[:, ts(j, 512)],
                            wq_sb[:, k, ts(m, 128)],
                            x_sb[:, k, ts(j, 512)],
                            start=(k == 0),
                            stop=(k == QK_CHUNKS - 1),
                        )
                nc.vector.tensor_copy(out=qT_sb[:, m, :], in_=qps)

            for m in range(IN_CHUNKS):
                for half in range(2):
                    emit_kt(m, half)
            emit_qt(qT0_sb, x_tiles[0], 0, 0)
            qT1_sb = stream.tile([128, IN_CHUNKS, QB], f32r, tag="qt2", bufs=1,
                                 name="qT1")
            qT_tiles = [qT0_sb, qT1_sb]
            # remaining qT passes are deferred into qb0's attention loop (on
            # the B ring): (h, kc) -> emission closure
            deferred_qt = {
                (0, 6): lambda: emit_qt(qT0_sb, x_tiles[0], 1, 0, psB, "B"),
                (2, 6): lambda: emit_qt(qT1_sb, x_tiles[1], 0, 1, psB, "B"),
                (3, 6): lambda: emit_qt(qT1_sb, x_tiles[1], 1, 1, psB, "B"),
            }

            # ---- v [M, 4, 65]: per key-chunk rows; col 64 of each head = 1.0.
            # The 16 projection chunks are *deferred*: they are emitted inside
            # the first attention head's loop (on the B PSUM ring) where the
            # PE has exp-shadow slack and ctx is fully resident.
            v_sb = persist.tile([128, KC, H_PER_CORE, D + 1], f32r, tag="v")
            nc.vector.tensor_copy(
                out=v_sb[:, :, :, D : D + 1],
                in_=onesf.rearrange("p (a b c) -> p a b c", a=KC, b=H_PER_CORE),
            )

            def emit_v_chunk(kc):
                vps = psB.tile([128, IN_PER_CORE], f32, tag="B", name=f"vps{kc}")
                for k in range(CK_CHUNKS):
                    nc.tensor.matmul(
                        vps,
                        ctx_sb[:, k, ts(kc, 128)],
                        wv_sb[:, k, :],
                        start=(k == 0),
                        stop=(k == CK_CHUNKS - 1),
                    )
                nc.vector.tensor_copy(
                    out=v_sb[:, kc, :, 0:D],
                    in_=vps.rearrange("p (h d) -> p h d", h=H_PER_CORE),
                )

            # ---- per query-block pipeline ----
            # final-projection chunks of the previous block are interleaved
            # into the (ACT-bound) attention loop of the current block so the
            # PE does them in exp shadows instead of an ACT-idle stretch.
            def emit_final_chunk(qb, qm, ot_all):
                ops = psB.tile([128, Q_DIM], f32, tag="B", name=f"ops{qb}{qm}")
                for t in range(IN_CHUNKS):
                    for j in range(2):
                        nc.tensor.matmul(
                            ops[:, ts(j, 512)],
                            ot_all[:, t, ts(qm, 128)],
                            wo_sb[:, t, ts(j, 512)],
                            start=(t == 0),
                            stop=(t == IN_CHUNKS - 1),
                        )
                ost = stream.tile([128, Q_DIM], f32, tag="ost", bufs=2,
                                  name=f"ost{qb}{qm}")
                nc.vector.tensor_copy(out=ost, in_=ops)
                nc.gpsimd.dma_start(
                    out=out_d[ds(qb * QB + qm * 128, 128), :], in_=ost
                )

            prev_final = None  # (qb, ot_all) awaiting final projection
            v_queue = list(range(KC))  # v chunks not yet emitted
            for qb in range(N_QB):
                final_queue = (
                    [(prev_final[0], qm, prev_final[1]) for qm in range(QB // 128)]
                    if prev_final is not None
                    else []
                )

                qT_sb = qT_tiles[qb]

                # attention output (normalized), [128, 2, QB] inner-major
                ot_all = stream.tile([128, IN_CHUNKS, QB], f32r, tag="otall",
                                     bufs=2, name=f"otall{qb}")

                # Software-pipelined attention: emit S(kc) before PV(kc-1) so
                # the PE queue never head-of-line blocks on exp.
                pending = None  # (h, kc, pt, ot_ps)

                def emit_pv(p):
                    h, kc, pt, ot_ps = p
                    for j in range(2):
                        nc.tensor.matmul(
                            ot_ps[:, ts(j, 512)],
                            v_sb[:, kc, h, :],
                            pt[:, ts(j, 512)],
                            start=(kc == 0),
                            stop=(kc == KC - 1),
                        )

                def emit_normalize(h, ot_ps):
                    t, po = h // 2, (h % 2) * D
                    ot_raw = stream.tile([D + 1, QB], f32r, tag="otraw", bufs=2,
                                         name=f"otraw{qb}{h}")
                    nc.vector.tensor_copy(out=ot_raw, in_=ot_ps)
                    bc_ps = psB.tile([D, QB], f32, tag="B", name=f"bc{qb}{h}")
                    for j in range(2):
                        nc.tensor.matmul(
                            bc_ps[:, ts(j, 512)],
                            ones_sb[D : D + 1, :],
                            ot_raw[D : D + 1, ts(j, 512)],
                            start=True,
                            stop=True,
                        )
                    nc.vector.reciprocal(out=bc_ps, in_=bc_ps)
                    nc.vector.tensor_mul(
                        out=ot_all[po : po + D, t, :],
                        in0=ot_raw[0:D, :],
                        in1=bc_ps,
                    )

                for h in range(H_PER_CORE):
                    t, po = h // 2, (h % 2) * D
                    ot_ps = psB.tile([D + 1, QB], f32, tag="B", name=f"ot{qb}{h}")
                    for kc in range(KC):
                        st = psA.tile([128, QB], f32, tag="A", name=f"st{qb}{h}{kc}")
                        for j in range(2):
                            nc.tensor.matmul(
                                st[:, ts(j, 512)],
                                kT_sb[po : po + D, t, ts(kc, 128)],
                                qT_sb[po : po + D, t, ts(j, 512)],
                                start=True,
                                stop=True,
                            )
                        if pending is not None:
                            emit_pv(pending)
                            if pending[1] == KC - 1:  # last chunk of a head
                                emit_normalize(pending[0], pending[3])
                        pt = stream.tile([128, QB], f32r, tag="pt", bufs=2,
                                         name=f"pt{qb}{h}{kc}")
                        nc.scalar.activation(out=pt, in_=st, func=Exp, scale=SCALE)
                        pending = (h, kc, pt, ot_ps)
                        # v chunks stay two key-chunks ahead of the PV stream
                        while v_queue and len(v_queue) > KC - 2 * (kc + 1):
                            emit_v_chunk(v_queue.pop(0))
                        if qb == 0 and (h, kc) in deferred_qt:
                            deferred_qt.pop((h, kc))()
                        # two previous-block final chunks per head, mid-loop
                        if final_queue and kc in (6, 12):
                            emit_final_chunk(*final_queue.pop(0))
                            emit_final_chunk(*final_queue.pop(0))

                # flush the last head of this block
                emit_pv(pending)
                emit_normalize(pending[0], pending[3])
                pending = None
                for fc in final_queue:
                    emit_final_chunk(*fc)
                prev_final = (qb, ot_all)

            # final projection of the last block
            for qm in range(QB // 128):
                emit_final_chunk(prev_final[0], qm, prev_final[1])

    nc.finalize()
    return nc


def _get_nc():
    global _CACHED_NC
    if _CACHED_NC is None:
        _CACHED_NC = _build_bass()
    return _CACHED_NC


def _numpy_fallback(x, context, mask, Wq, Wk, Wv, Wout, bout):
    q = (x @ Wq.T).reshape(B, N, H, D)
    k = (context @ Wk.T).reshape(B, M, H, D)
    v = (context @ Wv.T).reshape(B, M, H, D)
    sim = np.einsum("bnhd,bmhd->bhnm", q, k) * SCALE
    sim = np.where(mask[:, None, None, :], sim, -np.finfo(np.float32).max)
    sim -= sim.max(axis=-1, keepdims=True)
    attn = np.exp(sim)
    attn /= attn.sum(axis=-1, keepdims=True)
    out = np.einsum("bhnm,bmhd->bnhd", attn, v).reshape(B, N, INNER)
    return (out @ Wout.T + bout).astype(np.float32)


def kernel(x, context, mask, Wq, Wk, Wv, Wout, bout, _want_results=False):
    x = np.asarray(x, dtype=np.float32)
    context = np.asarray(context, dtype=np.float32)
    mask = np.asarray(mask)
    Wq = np.asarray(Wq, dtype=np.float32)
    Wk = np.asarray(Wk, dtype=np.float32)
    Wv = np.asarray(Wv, dtype=np.float32)
    Wout = np.asarray(Wout, dtype=np.float32)
    bout = np.asarray(bout, dtype=np.float32)

    if not mask.all():
        return _numpy_fallback(x, context, mask, Wq, Wk, Wv, Wout, bout)

    from concourse.bass_utils import run_bass_kernel_spmd

    in_maps = []
    for c in range(N_CORES):
        b, hg = c // 2, c % 2
        sl = slice(hg * IN_PER_CORE, (hg + 1) * IN_PER_CORE)
        in_maps.append(
            {
                "xT": np.ascontiguousarray(x[b].T),
                "cT": np.ascontiguousarray(context[b].T),
                "wq": np.ascontiguousarray(Wq[sl, :].T),
                "wk": np.ascontiguousarray(Wk[sl, :].T),
                "wv": np.ascontiguousarray(Wv[sl, :].T),
                "wo": np.ascontiguousarray(Wout[:, sl].T),
            }
        )

    res = run_bass_kernel_spmd(_get_nc(), in_maps, core_ids=list(range(N_CORES)))

    out = np.empty((B, N, Q_DIM), dtype=np.float32)
    for b in range(B):
        out[b] = res.results[2 * b]["out"] + res.results[2 * b + 1]["out"] + bout
    if _want_results:
        return out, res
    return out


# revision 18
# speedup vs baseline: 1.0122x; 1.0122x over previous
"""Trainium2 Bass kernel for CrossAttention (B=4, N=M=2048, H=8, D=64,
Q_DIM=1024, C_DIM=768).

Sharding over 8 cores: core c handles batch b = c//2 and head-group
hg = c%2 (4 heads, 256 inner dims).  Each core computes a *partial*
output projection (its 256 inner dims of the 512 total); the host sums
core pairs and adds the output bias.

Device-side layouts are all matmul-native (out = lhsT.T @ rhs):
  - qT/kT [inner, seq]: computed with weight-chunk stationary, xT/ctxT
    moving.  v [keys, inner] with ctxT-chunk stationary, WvT moving.
  - scores are computed transposed: S.T[keys, q] = kT_h contracted with
    qT_h over the head dim, so softmax's key-reduction is a matmul
    reduction and no on-chip transposes are needed anywhere.
  - V carries an appended ones-column, so the P.T@V matmul also yields the
    per-query softmax denominators (row 64 of the [65, q] accumulator).
  - denominators are broadcast across partitions with a K=1 ones matmul,
    reciprocal'd on VectorE, and folded into the PSUM->SBUF copy of the
    attention output.
  - all matmul inputs are float32r (single-pass FP22 multiply, fp32
    accumulate) for 4x tensor-engine throughput vs true fp32.
  - the attention inner loop is software-pipelined at emission so the
    next chunk's score matmuls sit ahead of the current chunk's PV
    matmul in the PE queue (PV waits on exp; the scores don't).

The attention mask in this problem is all-True; if a mask with False
entries is ever passed, kernel() falls back to a numpy reference.
"""

import numpy as np

B, N, M = 4, 2048, 2048
Q_DIM, C_DIM, H, D = 1024, 768, 8, 64
INNER = H * D  # 512
SCALE = D ** -0.5

N_CORES = 8
H_PER_CORE = 4          # heads per core
IN_PER_CORE = H_PER_CORE * D  # 256 inner dims per core
QB = 1024               # query block
N_QB = N // QB          # 2
KC = M // 128           # 16 key chunks
QK_CHUNKS = Q_DIM // 128   # 8
CK_CHUNKS = C_DIM // 128   # 6
IN_CHUNKS = IN_PER_CORE // 128  # 2

_CACHED_NC = None


def _build_bass():
    import concourse.bass as bass
    import concourse.mybir as mybir
    import concourse.tile as tile
    from concourse import bacc

    f32r = mybir.dt.float32r
    f32 = mybir.dt.float32
    ts, ds = bass.ts, bass.ds
    Exp = mybir.ActivationFunctionType.Exp

    nc = bacc.Bacc("TRN2", target_bir_lowering=False)

    xT = nc.dram_tensor("xT", [Q_DIM, N], f32r, kind="ExternalInput")
    cT = nc.dram_tensor("cT", [C_DIM, M], f32r, kind="ExternalInput")
    wq = nc.dram_tensor("wq", [Q_DIM, IN_PER_CORE], f32r, kind="ExternalInput")
    wk = nc.dram_tensor("wk", [C_DIM, IN_PER_CORE], f32r, kind="ExternalInput")
    wv = nc.dram_tensor("wv", [C_DIM, IN_PER_CORE], f32r, kind="ExternalInput")
    wo = nc.dram_tensor("wo", [IN_PER_CORE, Q_DIM], f32r, kind="ExternalInput")
    out_d = nc.dram_tensor("out", [N, Q_DIM], f32, kind="ExternalOutput")

    with tile.TileContext(nc) as tc:
        with (
            tc.tile_pool(name="persist", bufs=1) as persist,
            tc.tile_pool(name="stream", bufs=2) as stream,
            tc.tile_pool(name="psA", bufs=2, space="PSUM") as psA,
            tc.tile_pool(name="psB", bufs=2, space="PSUM") as psB,
        ):
            # ---- resident weights / constants ----
            # DMA order matters: the kT phase needs wk + ctx first.
            wk_sb = persist.tile([128, CK_CHUNKS, IN_PER_CORE], f32r, tag="wk")
            nc.sync.dma_start(out=wk_sb, in_=wk.rearrange("(k p) n -> p k n", p=128))

            # row D used as the lhsT of the denominator-broadcast matmul; it
            # must sit at the same base partition as the denominator row
            # (partition D of the attention accumulator).  memset can't target
            # float32r, so fill an fp32 scratch and copy-convert.
            onesf = persist.tile([128, D], f32, tag="onesf")
            nc.vector.memset(onesf, 1.0)
            ones_sb = persist.tile([D + 1, D], f32r, tag="ones")
            nc.vector.tensor_copy(out=ones_sb, in_=onesf[0 : D + 1, :])

            # context resident (read by both kT and v phases); per-chunk DMAs
            # so the kT matmuls can start as soon as the first chunk lands.
            ctx_sb = persist.tile([128, CK_CHUNKS, M], f32r, tag="ctx")
            for k in range(CK_CHUNKS):
                nc.sync.dma_start(out=ctx_sb[:, k, :], in_=cT[ds(k * 128, 128), :])

            wq_sb = persist.tile([128, QK_CHUNKS, IN_PER_CORE], f32r, tag="wq")
            wq_r = wq.rearrange("(k p) n -> p k n", p=128)
            nc.sync.dma_start(out=wq_sb[:, :, 0:128], in_=wq_r[:, :, 0:128])

            # load order: x0 right after wq (gates the first exp), then wv
            # (gates the deferred v chunks), wo, and x1 last (needed ~100us in;
            # it waits on the x slot release semaphore at runtime anyway).
            def emit_x_load(qb, chunked=False):
                x_sb = stream.tile([128, QK_CHUNKS, QB], f32r, tag="x", bufs=1,
                                   name=f"x{qb}")
                if chunked:
                    # per-chunk DMAs so the first qT pass paces with arrival
                    for k in range(QK_CHUNKS):
                        nc.sync.dma_start(
                            out=x_sb[:, k, :],
                            in_=xT[ds(k * 128, 128), ds(qb * QB, QB)],
                        )
                else:
                    nc.sync.dma_start(
                        out=x_sb,
                        in_=xT[:, ds(qb * QB, QB)].rearrange("(k p) q -> p k q",
                                                             p=128),
                    )
                return x_sb

            x_tiles = [emit_x_load(0, chunked=True)]

            wv_sb = persist.tile([128, CK_CHUNKS, IN_PER_CORE], f32r, tag="wv")
            nc.sync.dma_start(out=wv_sb, in_=wv.rearrange("(k p) n -> p k n", p=128))
            wo_sb = persist.tile([128, IN_CHUNKS, Q_DIM], f32r, tag="wo")
            nc.sync.dma_start(out=wo_sb, in_=wo.rearrange("(t p) n -> p t n", p=128))
            nc.sync.dma_start(out=wq_sb[:, :, 128:256], in_=wq_r[:, :, 128:256])
            x_tiles.append(emit_x_load(1))

            # ---- kT [IN_PER_CORE, M] interleaved with qT(qb0) so the PE can
            # alternate between them as the ctx / x0 DMAs land (kps and qps
            # occupy the two A slots concurrently).
            kT_sb = persist.tile([128, IN_CHUNKS, M], f32r, tag="kt")
            qT0_sb = stream.tile([128, IN_CHUNKS, QB], f32r, tag="qt", bufs=1,
                                 name="qT0")

            def emit_kt(m, half):
                kps = psA.tile([128, 1024], f32, tag="A", name=f"kps{m}{half}")
                for k in range(CK_CHUNKS):
                    for j in range(2):
                        nc.tensor.matmul(
                            kps[:, ts(j, 512)],
                            wk_sb[:, k, ts(m, 128)],
                            ctx_sb[:, k, ds(half * 1024 + j * 512, 512)],
                            start=(k == 0),
                            stop=(k == CK_CHUNKS - 1),
                        )
                nc.vector.tensor_copy(
                    out=kT_sb[:, m, ds(half * 1024, 1024)], in_=kps
                )

            def emit_qt(qT_sb, x_sb, m, qb, pool=None, tag="A"):
                pool = pool if pool is not None else psA
                qps = pool.tile([128, QB], f32, tag=tag, name=f"qps{qb}{m}")
                for k in range(QK_CHUNKS):
                    for j in range(2):
                        nc.tensor.matmul(
                            qps[:, ts(j, 512)],
                            wq_sb[:, k, ts(m, 128)],
                            x_sb[:, k, ts(j, 512)],
                            start=(k == 0),
                            stop=(k == QK_CHUNKS - 1),
                        )
                nc.vector.tensor_copy(out=qT_sb[:, m, :], in_=qps)

            for m in range(IN_CHUNKS):
                for half in range(2):
                    emit_kt(m, half)
            emit_qt(qT0_sb, x_tiles[0], 0, 0)
            qT1_sb = stream.tile([128, IN_CHUNKS, QB], f32r, tag="qt2", bufs=1,
                                 name="qT1")
            qT_tiles = [qT0_sb, qT1_sb]
            # remaining qT passes are deferred into qb0's attention loop (on
            # the B ring): (h, kc) -> emission closure
            deferred_qt = {
                (0, 6): lambda: emit_qt(qT0_sb, x_tiles[0], 1, 0, psB, "B"),
                (2, 6): lambda: emit_qt(qT1_sb, x_tiles[1], 0, 1, psB, "B"),
                (3, 6): lambda: emit_qt(qT1_sb, x_tiles[1], 1, 1, psB, "B"),
            }

            # ---- v [M, 4, 65]: per key-chunk rows; col 64 of each head = 1.0.
            # The 16 projection chunks are *deferred*: they are emitted inside
            # the first attention head's loop (on the B PSUM ring) where the
            # PE has exp-shadow slack and ctx is fully resident.
            v_sb = persist.tile([128, KC, H_PER_CORE, D + 1], f32r, tag="v")
            nc.vector.tensor_copy(
                out=v_sb[:, :, :, D : D + 1],
                in_=onesf.rearrange("p (a b c) -> p a b c", a=KC, b=H_PER_CORE),
            )

            def emit_v_chunk(kc):
                vps = psB.tile([128, IN_PER_CORE], f32, tag="B", name=f"vps{kc}")
                for k in range(CK_CHUNKS):
                    nc.tensor.matmul(
                        vps,
                        ctx_sb[:, k, ts(kc, 128)],
                        wv_sb[:, k, :],
                        start=(k == 0),
                        stop=(k == CK_CHUNKS - 1),
                    )
                nc.vector.tensor_copy(
                    out=v_sb[:, kc, :, 0:D],
                    in_=vps.rearrange("p (h d) -> p h d", h=H_PER_CORE),
                )

            # ---- per query-block pipeline ----
            # final-projection chunks of the previous block are interleaved
            # into the (ACT-bound) attention loop of the current block so the
            # PE does them in exp shadows instead of an ACT-idle stretch.
            def emit_final_chunk(qb, qm, ot_all, on_act=False):
                ops = psB.tile([128, Q_DIM], f32, tag="B", name=f"ops{qb}{qm}")
                for t in range(IN_CHUNKS):
                    for j in range(2):
                        nc.tensor.matmul(
                            ops[:, ts(j, 512)],
                            ot_all[:, t, ts(qm, 128)],
                            wo_sb[:, t, ts(j, 512)],
                            start=(t == 0),
                            stop=(t == IN_CHUNKS - 1),
                        )
                ost = stream.tile([128, Q_DIM], f32, tag="ost", bufs=2,
                                  name=f"ost{qb}{qm}")
                # tail finals run after the last exp: ScalarE is idle there,
                # VectorE is not (it owns the last normalize chain)
                if on_act:
                    nc.scalar.copy(out=ost, in_=ops)
                else:
                    nc.vector.tensor_copy(out=ost, in_=ops)
                nc.gpsimd.dma_start(
                    out=out_d[ds(qb * QB + qm * 128, 128), :], in_=ost
                )

            prev_final = None  # (qb, ot_all) awaiting final projection
            v_queue = list(range(KC))  # v chunks not yet emitted
            for qb in range(N_QB):
                final_queue = (
                    [(prev_final[0], qm, prev_final[1]) for qm in range(QB // 128)]
                    if prev_final is not None
                    else []
                )

                qT_sb = qT_tiles[qb]

                # attention output (normalized), [128, 2, QB] inner-major
                ot_all = stream.tile([128, IN_CHUNKS, QB], f32r, tag="otall",
                                     bufs=2, name=f"otall{qb}")

                # Software-pipelined attention: emit S(kc) before PV(kc-1) so
                # the PE queue never head-of-line blocks on exp.
                pending = None  # (h, kc, pt, ot_ps)

                def emit_pv(p):
                    h, kc, pt, ot_ps = p
                    for j in range(2):
                        nc.tensor.matmul(
                            ot_ps[:, ts(j, 512)],
                            v_sb[:, kc, h, :],
                            pt[:, ts(j, 512)],
                            start=(kc == 0),
                            stop=(kc == KC - 1),
                        )

                def emit_normalize(h, ot_ps):
                    t, po = h // 2, (h % 2) * D
                    ot_raw = stream.tile([D + 1, QB], f32r, tag="otraw", bufs=2,
                                         name=f"otraw{qb}{h}")
                    nc.vector.tensor_copy(out=ot_raw, in_=ot_ps)
                    bc_ps = psB.tile([D, QB], f32, tag="B", name=f"bc{qb}{h}")
                    for j in range(2):
                        nc.tensor.matmul(
                            bc_ps[:, ts(j, 512)],
                            ones_sb[D : D + 1, :],
                            ot_raw[D : D + 1, ts(j, 512)],
                            start=True,
                            stop=True,
                        )
                    nc.vector.reciprocal(out=bc_ps, in_=bc_ps)
                    nc.vector.tensor_mul(
                        out=ot_all[po : po + D, t, :],
                        in0=ot_raw[0:D, :],
                        in1=bc_ps,
                    )

                for h in range(H_PER_CORE):
                    t, po = h // 2, (h % 2) * D
                    ot_ps = psB.tile([D + 1, QB], f32, tag="B", name=f"ot{qb}{h}")
                    for kc in range(KC):
                        st = psA.tile([128, QB], f32, tag="A", name=f"st{qb}{h}{kc}")
                        for j in range(2):
                            nc.tensor.matmul(
                                st[:, ts(j, 512)],
                                kT_sb[po : po + D, t, ts(kc, 128)],
                                qT_sb[po : po + D, t, ts(j, 512)],
                                start=True,
                                stop=True,
                            )
                        if pending is not None:
                            emit_pv(pending)
                            if pending[1] == KC - 1:  # last chunk of a head
                                emit_normalize(pending[0], pending[3])
                        pt = stream.tile([128, QB], f32r, tag="pt", bufs=3,
                                         name=f"pt{qb}{h}{kc}")
                        nc.scalar.activation(out=pt, in_=st, func=Exp, scale=SCALE)
                        pending = (h, kc, pt, ot_ps)
                        # v chunks stay two key-chunks ahead of the PV stream
                        while v_queue and len(v_queue) > KC - 2 * (kc + 1):
                            emit_v_chunk(v_queue.pop(0))
                        if qb == 0 and (h, kc) in deferred_qt:
                            deferred_qt.pop((h, kc))()
                        # two previous-block final chunks per head, mid-loop
                        if final_queue and kc in (6, 12):
                            emit_final_chunk(*final_queue.pop(0))
                            emit_final_chunk(*final_queue.pop(0))

                # flush the last head of this block
                emit_pv(pending)
                emit_normalize(pending[0], pending[3])
                pending = None
                for fc in final_queue:
                    emit_final_chunk(*fc)
                prev_final = (qb, ot_all)

            # final projection of the last block
            for qm in range(QB // 128):
                emit_final_chunk(prev_final[0], qm, prev_final[1], on_act=True)

    nc.finalize()
    return nc


def _get_nc():
    global _CACHED_NC
    if _CACHED_NC is None:
        _CACHED_NC = _build_bass()
    return _CACHED_NC


def _numpy_fallback(x, context, mask, Wq, Wk, Wv, Wout, bout):
    q = (x @ Wq.T).reshape(B, N, H, D)
    k = (context @ Wk.T).reshape(B, M, H, D)
    v = (context @ Wv.T).reshape(B, M, H, D)
    sim = np.einsum("bnhd,bmhd->bhnm", q, k) * SCALE
    sim = np.where(mask[:, None, None, :], sim, -np.finfo(np.float32).max)
    sim -= sim.max(axis=-1, keepdims=True)
    attn = np.exp(sim)
    attn /= attn.sum(axis=-1, keepdims=True)
    out = np.einsum("bhnm,bmhd->bnhd", attn, v).reshape(B, N, INNER)
    return (out @ Wout.T + bout).astype(np.float32)


def kernel(x, context, mask, Wq, Wk, Wv, Wout, bout, _want_results=False):
    x = np.asarray(x, dtype=np.float32)
    context = np.asarray(context, dtype=np.float32)
    mask = np.asarray(mask)
    Wq = np.asarray(Wq, dtype=np.float32)
    Wk = np.asarray(Wk, dtype=np.float32)
    Wv = np.asarray(Wv, dtype=np.float32)
    Wout = np.asarray(Wout, dtype=np.float32)
    bout = np.asarray(bout, dtype=np.float32)

    if not mask.all():
        return _numpy_fallback(x, context, mask, Wq, Wk, Wv, Wout, bout)

    from concourse.bass_utils import run_bass_kernel_spmd

    in_maps = []
    for c in range(N_CORES):
        b, hg = c // 2, c % 2
        sl = slice(hg * IN_PER_CORE, (hg + 1) * IN_PER_CORE)
        in_maps.append(
            {
                "xT": np.ascontiguousarray(x[b].T),
                "cT": np.ascontiguousarray(context[b].T),
                "wq": np.ascontiguousarray(Wq[sl, :].T),
                "wk": np.ascontiguousarray(Wk[sl, :].T),
                "wv": np.ascontiguousarray(Wv[sl, :].T),
                "wo": np.ascontiguousarray(Wout[:, sl].T),
            }
        )

    res = run_bass_kernel_spmd(_get_nc(), in_maps, core_ids=list(range(N_CORES)))

    out = np.empty((B, N, Q_DIM), dtype=np.float32)
    for b in range(B):
        out[b] = res.results[2 * b]["out"] + res.results[2 * b + 1]["out"] + bout
    if _want_results:
        return out, res
    return out
